# revision 1
# baseline (speedup 1.0000x reference)
"""Trainium2 Bass kernel for multiplicative-tril-mask attention (8 NeuronCores).

Problem: B=4, T=2048, DIN=DOUT=1024
  q = x @ Wq.T ; k = x @ Wk.T ; v = x @ Wv.T
  attn = (q @ k.T) * tril_ones        # multiplicative mask: masked logits -> 0
  attn = softmax(attn / sqrt(T))      # masked entries contribute exp(0)=1
  out = attn @ v

Sharding (one SPMD program on 8 cores, 2 cores per batch):
 - Balanced causal query split: parity-0 cores own queries [0,512)u[1536,2048),
   parity-1 cores own [512,1536). Each 512-query slot has a fixed key-tile
   window (slot0: k<1024, slot1: k<2048) so the program is identical across
   cores; per-core behavior differs only through input data (packed query
   columns xTq, mask-generator qmi, suffix rows ssuf).
 - K/V tensor-parallel: each core projects only its half of K^T and V; halves
   are exchanged with 2-core AllGathers over DRAM bounce buffers while the PE
   computes Q^T and both slots' score matrices (scores never touch V).
 - Keys beyond a query-subtile's window are all masked (each contributes
   exp(0)*V[k] to the numerator and 1 to the denominator): handled by a
   host-precomputed suffix column-sum row (ssuf) broadcast via a K=1 matmul,
   plus a compile-time constant in the denominator.

Layouts are chosen so NO on-chip transposes are needed:
  xT[d,t], wT[d,e] host-pretransposed; Q^T/K^T [e,t] (e on partitions);
  scores^T[k,q] = matmul(lhsT=K^T, rhs=Q^T); p^T = exp(masked scores^T) is
  directly the lhsT of the PV matmul with natural-layout V[t,e] as rhs.
  Logits are bounded (~[-1.3, 1.3]), so exp needs no max-subtraction and the
  denominator comes from a ones-column matmul.
Compute dtype bf16 (PE runs fp32 at 1/4 rate), accumulation + softmax in f32.
Measured: ~205-215 us fast-phase on silicon (chip power-state dependent;
best 204.5 us), rel err 2.9e-3 vs the f32 reference.
"""

import os
import sys

sys.path.insert(0, "/opt/trn_rl_repo")

import numpy as np
import ml_dtypes

import concourse.bass as bass
import concourse.tile as tile
from concourse import bacc, mybir
from concourse import bass_utils

bass_utils.upload_artifacts = lambda tmpdir: "local://" + tmpdir

B, T, D = 4, 2048, 1024
N_CORES = 8
NDT = D // 128
NET = D // 128
NKT_ALL = T // 128
HALF = T // 2  # 1024

SLOT_STARTS = [[0, 1536], [512, 1024]]
NKT = [8, 16]
DENC = [float(T - 128 * NKT[0]), float(T - 128 * NKT[1])]
SCALE = 1.0 / float(np.sqrt(np.float32(T)))

GROUPS = [[0, 1], [2, 3], [4, 5], [6, 7]]

BF = mybir.dt.bfloat16
F32 = mybir.dt.float32
bf16 = ml_dtypes.bfloat16

_cache = {}
LAST_RESULT = None


def _build():
    nc = bacc.Bacc("TRN2", target_bir_lowering=False, debug=False, num_devices=N_CORES)

    xTh_d = nc.dram_tensor("xTh", [D, HALF], BF, kind="ExternalInput")
    xTq_d = nc.dram_tensor("xTq", [D, 1024], BF, kind="ExternalInput")
    wq_d = nc.dram_tensor("wq", [D, D], BF, kind="ExternalInput")
    wk_d = nc.dram_tensor("wk", [D, D], BF, kind="ExternalInput")
    wv_d = nc.dram_tensor("wv", [D, D], BF, kind="ExternalInput")
    qmi_d = nc.dram_tensor("qmi", [2, 128, 512], F32, kind="ExternalInput")
    ssuf_d = nc.dram_tensor("ssuf", [1, 8 * D], BF, kind="ExternalInput")
    out_d = nc.dram_tensor("out", [1024, D], F32, kind="ExternalOutput")

    xTh = xTh_d.ap()
    xTq = xTq_d.ap()
    qmi_ap = qmi_d.ap()
    out_ap = out_d.ap()

    Exp = mybir.ActivationFunctionType.Exp

    with tile.TileContext(nc) as tc:
        with (
            tc.tile_pool(name="actpool", bufs=1) as actpool,
            tc.tile_pool(name="cpool", bufs=1) as cpool,
            tc.tile_pool(name="drpool", bufs=1, space="DRAM") as drpool,
            tc.tile_pool(name="ps_big", bufs=6, space="PSUM") as ps_big,
            tc.tile_pool(name="ps_small", bufs=2, space="PSUM") as ps_small,
        ):
            # ---- constants ----
            ones_col = cpool.tile([128, 1], BF)
            nc.vector.memset(ones_col[:], 1.0)
            ones_row = cpool.tile([1, 128], BF)
            nc.vector.memset(ones_row[:], 1.0)
            one11 = cpool.tile([1, 1], F32)
            nc.vector.memset(one11[:], 1.0)

            qmi = cpool.tile([128, 2, 512], F32)
            for j in range(2):
                nc.scalar.dma_start(qmi[:, j, :], qmi_ap[j])

            # persistent activations
            QT = actpool.tile([128, NET, 1024], BF, tag="qt")
            KT = actpool.tile([128, NET, T], BF, tag="kt")
            V = actpool.tile([128, NKT_ALL, D], BF, tag="v")
            # host-precomputed suffix rows: row r=4j+qs holds colsum of V
            # over k >= 128*win(j,qs); all on partition 0
            ssuf = cpool.tile([1, 8 * D], BF)
            nc.scalar.dma_start(ssuf[:], ssuf_d.ap())

            # DRAM bounce buffers for collectives
            kbounce = drpool.tile([128, NET * HALF], BF, name="kbounce")
            kg = drpool.tile([256, NET * HALF], BF, name="kg")
            vbounce = drpool.tile([128, 8 * D], BF, name="vbounce")
            vg = drpool.tile([256, 8 * D], BF, name="vg")

            # ---- phase A ----
            with (
                tc.tile_pool(name="xpool", bufs=1) as xpool,
                tc.tile_pool(name="wpool", bufs=2) as wpool,
                tc.tile_pool(name="stpool", bufs=16) as stpool,
            ):
                # half-tile DMA order: the first K group (c=0, all et) needs
                # only wk (full) + xh columns 0:512, so land those first
                wk_t = wpool.tile([128, NDT, D], BF, tag="w")
                xh_all = xpool.tile([128, NDT, HALF], BF, tag="xh")
                for dt in range(NDT):
                    nc.sync.dma_start(
                        wk_t[:, dt, 0:512],
                        wk_d.ap()[128 * dt : 128 * (dt + 1), 0:512],
                    )
                    nc.sync.dma_start(
                        xh_all[:, dt, 0:512],
                        xTh[128 * dt : 128 * (dt + 1), 0:512],
                    )
                for dt in range(NDT):
                    nc.sync.dma_start(
                        wk_t[:, dt, 512:1024],
                        wk_d.ap()[128 * dt : 128 * (dt + 1), 512:1024],
                    )
                    nc.sync.dma_start(
                        xh_all[:, dt, 512:1024],
                        xTh[128 * dt : 128 * (dt + 1), 512:1024],
                    )
                wv_t = wpool.tile([128, NDT, D], BF, tag="w")
                for dt in range(NDT):
                    nc.sync.dma_start(
                        wv_t[:, dt, :], wv_d.ap()[128 * dt : 128 * (dt + 1), :]
                    )
                wq_t = wpool.tile([128, NDT, D], BF, tag="w")
                xq_all = xpool.tile([128, NDT, 1024], BF, tag="xq")
                for dt in range(NDT):
                    nc.sync.dma_start(
                        wq_t[:, dt, :], wq_d.ap()[128 * dt : 128 * (dt + 1), :]
                    )
                    nc.sync.dma_start(
                        xq_all[:, dt, :], xTq[128 * dt : 128 * (dt + 1), :]
                    )


                # K^T own half -> bounce (c outer: c=0 runs on first-half DMAs)
                for c in range(2):
                    for et in range(NET):
                        ps = ps_big.tile([128, 512], F32, tag="big", name="ps")
                        for dt in range(NDT):
                            nc.tensor.matmul(
                                ps[:],
                                wk_t[:, dt, 128 * et : 128 * (et + 1)],
                                xh_all[:, dt, 512 * c : 512 * (c + 1)],
                                start=(dt == 0),
                                stop=(dt == NDT - 1),
                            )
                        st = stpool.tile([128, 512], BF, tag="st", name="st")
                        nc.vector.tensor_copy(st[:], ps[:])
                        nc.scalar.dma_start(
                            kbounce[:, HALF * et + 512 * c : HALF * et + 512 * (c + 1)],
                            st[:],
                        )
                nc.gpsimd.collective_compute(
                    "AllGather",
                    mybir.AluOpType.bypass,
                    replica_groups=GROUPS,
                    ins=[kbounce.opt()],
                    outs=[kg.opt()],
                )
                # readback gathered K^T (sync queue is idle by now; bounce
                # outs live on gpsimd, exps own the scalar engine)
                for h in range(2):
                    for et in range(NET):
                        nc.sync.dma_start(
                            KT[:, et, HALF * h : HALF * (h + 1)],
                            kg[128 * h : 128 * (h + 1), HALF * et : HALF * (et + 1)],
                        )

                # V own half (8 k-tiles) -> bounce
                for i in range(8):
                    for ec in range(2):
                        ps = ps_big.tile([128, 512], F32, tag="big", name="ps")
                        for dt in range(NDT):
                            nc.tensor.matmul(
                                ps[:],
                                xh_all[:, dt, 128 * i : 128 * (i + 1)],
                                wv_t[:, dt, 512 * ec : 512 * (ec + 1)],
                                start=(dt == 0),
                                stop=(dt == NDT - 1),
                            )
                        st = stpool.tile([128, 512], BF, tag="st", name="st")
                        nc.vector.tensor_copy(st[:], ps[:])
                        nc.scalar.dma_start(
                            vbounce[:, D * i + 512 * ec : D * i + 512 * (ec + 1)],
                            st[:],
                        )
                nc.gpsimd.collective_compute(
                    "AllGather",
                    mybir.AluOpType.bypass,
                    replica_groups=GROUPS,
                    ins=[vbounce.opt()],
                    outs=[vg.opt()],
                )
                for h in range(2):
                    for i in range(8):
                        nc.sync.dma_start(
                            V[:, 8 * h + i, :],
                            vg[128 * h : 128 * (h + 1), D * i : D * (i + 1)],
                        )

                # Q^T (own queries) -- fills the PE while CC(V) is in flight
                for et in range(NET):
                    for c in range(2):
                        ps = ps_big.tile([128, 512], F32, tag="big", name="ps")
                        for dt in range(NDT):
                            nc.tensor.matmul(
                                ps[:],
                                wq_t[:, dt, 128 * et : 128 * (et + 1)],
                                xq_all[:, dt, 512 * c : 512 * (c + 1)],
                                start=(dt == 0),
                                stop=(dt == NDT - 1),
                            )
                        nc.vector.tensor_copy(QT[:, et, 512 * c : 512 * (c + 1)], ps[:])

            # ---- phase B (identical to V1) ----
            with (
                tc.tile_pool(name="ppool", bufs=2) as ppool,
                tc.tile_pool(name="mpool", bufs=3) as mpool,
                tc.tile_pool(name="spool", bufs=2) as spool,
                tc.tile_pool(name="opool", bufs=3) as opool,
            ):
                pTs, rrows = {}, {}
                for j in (1, 0):
                    ktj = NKT[j]
                    mask_from = 0 if j == 0 else 8

                    pT = ppool.tile([128, NKT_ALL, 512], BF, tag="pT", name="pT")
                    pTs[j] = pT
                    dps = ps_small.tile([1, 512], F32, tag="small", name="dps", bufs=1)
                    for kt in range(ktj):
                        zps = ps_big.tile([128, 512], F32, tag="big", name="zps")
                        for et in range(NET):
                            nc.tensor.matmul(
                                zps[:],
                                KT[:, et, 128 * kt : 128 * (kt + 1)],
                                QT[:, et, 512 * j : 512 * (j + 1)],
                                start=(et == 0),
                                stop=(et == NET - 1),
                            )
                        if kt >= mask_from:
                            mt = mpool.tile([128, 512], F32, tag="mask", name="mt")
                            nc.vector.tensor_scalar(
                                mt[:],
                                qmi[:, j, :],
                                float(128 * kt),
                                None,
                                op0=mybir.AluOpType.is_ge,
                            )
                            nc.vector.tensor_mul(zps[:], zps[:], mt[:])
                        nc.scalar.activation(pT[:, kt, :], zps[:], Exp, scale=SCALE)
                        # denominator, lagged 2 groups behind the scores
                        # stream so the PE never waits on the exp chain
                        if kt >= 2:
                            nc.tensor.matmul(
                                dps[:],
                                ones_col[:],
                                pT[:, kt - 2, :],
                                start=(kt == 2),
                                stop=False,
                            )
                    for kt in (ktj - 2, ktj - 1):
                        nc.tensor.matmul(
                            dps[:],
                            ones_col[:],
                            pT[:, kt, :],
                            start=False,
                            stop=(kt == ktj - 1),
                        )
                    drow = spool.tile([1, 512], F32, tag="drow", name="drow")
                    nc.vector.tensor_scalar_add(drow[:], dps[:], DENC[j])
                    rrow = spool.tile([1, 512], F32, tag="rrow", name="rrow")
                    nc.vector.reciprocal(rrow[:], drow[:])
                    rrows[j] = rrow

                for j in (1, 0):
                    ktj = NKT[j]
                    pT = pTs[j]
                    rrow = rrows[j]
                    for qs in range(4):
                        win = min(NKT[j] - 3 + qs, NKT_ALL)  # 5+qs / 13+qs
                        npss = []
                        for ec in range(2):
                            nps = ps_big.tile([128, 512], F32, tag="big", name="nps")
                            for kt in range(win):
                                nc.tensor.matmul(
                                    nps[:],
                                    pT[:, kt, 128 * qs : 128 * (qs + 1)],
                                    V[:, kt, 512 * ec : 512 * (ec + 1)],
                                    start=(kt == 0),
                                    stop=(kt == win - 1 and win == NKT_ALL),
                                )
                            if win < NKT_ALL:
                                r = 4 * j + qs
                                nc.tensor.matmul(
                                    nps[:],
                                    ones_row[:],
                                    ssuf[0:1, D * r + 512 * ec : D * r + 512 * (ec + 1)],
                                    start=False,
                                    stop=True,
                                )
                            npss.append(nps)
                        rps = ps_small.tile([128, 1], F32, tag="rden", name="rps", bufs=1)
                        nc.tensor.matmul(
                            rps[:], rrow[0:1, 128 * qs : 128 * (qs + 1)], one11[:]
                        )
                        rcol = spool.tile([128, 1], F32, tag="rcol", name="rcol")
                        nc.vector.tensor_copy(rcol[:], rps[:])
                        for ec in range(2):
                            nps = npss[ec]
                            ot = opool.tile([128, 512], F32, tag="out", name="ot")
                            nc.vector.tensor_scalar_mul(ot[:], nps[:], rcol[:])
                            nc.sync.dma_start(
                                out_ap[
                                    512 * j + 128 * qs : 512 * j + 128 * (qs + 1),
                                    512 * ec : 512 * (ec + 1),
                                ],
                                ot[:],
                            )

    nc.compile()
    return nc


def get_nc():
    if "nc" not in _cache:
        _cache["nc"] = _build()
    return _cache["nc"]


def make_in_maps(x, Wq, Wk, Wv):
    x = np.asarray(x, np.float32)
    wqT = np.ascontiguousarray(np.asarray(Wq, np.float32).T).astype(bf16)
    wkT = np.ascontiguousarray(np.asarray(Wk, np.float32).T).astype(bf16)
    wvT = np.ascontiguousarray(np.asarray(Wv, np.float32).T).astype(bf16)

    qmis = []
    for p in range(2):
        qmi = np.empty((2, 128, 512), np.float32)
        for j in range(2):
            s = SLOT_STARTS[p][j]
            qmi[j] = (s + np.arange(512, dtype=np.float32))[None, :] - np.arange(
                128, dtype=np.float32
            )[:, None]
        qmis.append(qmi)

    wv32 = np.asarray(Wv, np.float32)
    ssufs = []
    for b in range(B):
        rows = np.zeros((8, D), np.float32)
        for j in range(2):
            for qs in range(4):
                win = min(NKT[j] - 3 + qs, NKT_ALL)
                if win < NKT_ALL:
                    cs = x[b][128 * win :, :].sum(axis=0, dtype=np.float32)
                    rows[4 * j + qs] = cs @ wv32.T
        ssufs.append(rows.reshape(1, 8 * D).astype(bf16))

    in_maps = []
    for core in range(N_CORES):
        b, p = core // 2, core % 2
        xt = np.ascontiguousarray(x[b].T).astype(bf16)  # [D, T]
        xh = np.ascontiguousarray(xt[:, HALF * p : HALF * (p + 1)])
        cols = []
        for j in range(2):
            s = SLOT_STARTS[p][j]
            cols.append(xt[:, s : s + 512])
        xq = np.ascontiguousarray(np.concatenate(cols, axis=1))
        in_maps.append(
            {
                "xTh": xh,
                "xTq": xq,
                "wq": wqT,
                "wk": wkT,
                "wv": wvT,
                "qmi": qmis[p],
                "ssuf": ssufs[b],
            }
        )
    return in_maps


def assemble(results):
    full = np.empty((B, T, D), np.float32)
    for core in range(N_CORES):
        b, p = core // 2, core % 2
        o = results[core]["out"]
        for j in range(2):
            s = SLOT_STARTS[p][j]
            full[b, s : s + 512, :] = o[512 * j : 512 * (j + 1), :]
    return full


def kernel(x, Wq, Wk, Wv):
    global LAST_RESULT
    nc = get_nc()
    in_maps = make_in_maps(x, Wq, Wk, Wv)
    res = bass_utils.run_bass_kernel_spmd(nc, in_maps, core_ids=list(range(N_CORES)))
    LAST_RESULT = res
    return assemble(res.results)



# revision 2
# speedup vs baseline: 1.1157x; 1.1157x over previous
"""Trainium2 Bass kernel V2 for multiplicative-tril-mask attention (8 cores).

Problem: B=4, T=2048, DIN=DOUT=1024
  q = x @ Wq.T ; k = x @ Wk.T ; v = x @ Wv.T
  attn = (q @ k.T) * tril_ones        # multiplicative mask: masked logits -> 0
  attn = softmax(attn / sqrt(T))      # masked entries contribute exp(0)=1
  out = attn @ v

Key structural wins over V1:
 1. M-trick: scores = x (Wq^T Wk / sqrt(T)) x^T. M is host-precomputed, so
    the K projection AND the K all-gather disappear; scores contract the
    raw input xT (global key order, local to every core) against
    yT = M^T x (computed only for owned queries).
 2. Paired causal windows: core parity p owns global 128-query subtiles
    {2i+p}. Program slot o serves global subtile 2o+p -> baked windows
    w(o)=2o+2 k-tiles (total 72 vs V1's 84) for PV, and score groups
    g (owned subtiles 2g,2g+1; N=256 moving) with window 4g+4 (total 40
    k-tile-groups ~ 80 vs 96 tile-equivalents).
 3. Only one collective remains (V halves, 2-core AllGather via DRAM
    bounce), fully hidden behind yT + scores.
 4. bf16 output (host upcasts); suffix-masked keys handled by host
    precomputed ssuf rows folded into the PV PSUM chain (K=1 matmul).
"""

import os
import sys

sys.path.insert(0, "/opt/trn_rl_repo")

import numpy as np
import ml_dtypes

import concourse.bass as bass
import concourse.tile as tile
from concourse import bacc, mybir
from concourse import bass_utils

bass_utils.upload_artifacts = lambda tmpdir: "local://" + tmpdir

B, T, D = 4, 2048, 1024
N_CORES = 8
NDT = D // 128          # 8 contraction tiles
NKT_ALL = T // 128      # 16 key tiles
HALF = T // 2           # 1024

SCALE = 1.0 / float(np.sqrt(np.float32(T)))
GROUPS = [[0, 1], [2, 3], [4, 5], [6, 7]]

BF = mybir.dt.bfloat16
F8 = mybir.dt.float8e4
F32 = mybir.dt.float32
bf16 = ml_dtypes.bfloat16
f8e4 = ml_dtypes.float8_e4m3

# fp8 DoubleRow scores: HW-measured 1.63x over bf16 at this exact shape.
# y is scaled by YSC (folded into M on the host) so its rms ~0.47 sits in
# e4m3's normal range; the exp activation divides it back out.
FP8_SCORES = True
YSC = 64.0

_cache = {}
LAST_RESULT = None


def _w(o):          # PV window (k-tiles) for owned subtile slot o
    return 2 * o + 2


def _wsc(g):        # score window (k-tiles) for score group g
    return 4 * g + 4


def _build():
    nc = bacc.Bacc("TRN2", target_bir_lowering=False, debug=False, num_devices=N_CORES)

    SDT = F8 if FP8_SCORES else BF
    xT_d = nc.dram_tensor("xT", [D, T], SDT, kind="ExternalInput")
    xTh_d = nc.dram_tensor("xTh", [D, HALF], BF, kind="ExternalInput")
    xTq_d = nc.dram_tensor("xTq", [D, 1024], BF, kind="ExternalInput")
    M_d = nc.dram_tensor("M", [D, D], BF, kind="ExternalInput")
    wv_d = nc.dram_tensor("wv", [D, D], BF, kind="ExternalInput")
    qmi_d = nc.dram_tensor("qmi", [4, 128, 512], F32, kind="ExternalInput")
    ssuf_d = nc.dram_tensor("ssuf", [1, 8 * D], BF, kind="ExternalInput")
    out_d = nc.dram_tensor("out", [1024, D], BF, kind="ExternalOutput")

    xT_ap = xT_d.ap()
    xTh = xTh_d.ap()
    xTq = xTq_d.ap()
    qmi_ap = qmi_d.ap()
    out_ap = out_d.ap()

    Exp = mybir.ActivationFunctionType.Exp

    with tile.TileContext(nc) as tc:
        with (
            tc.tile_pool(name="actpool", bufs=1) as actpool,
            tc.tile_pool(name="cpool", bufs=1) as cpool,
            tc.tile_pool(name="drpool", bufs=1, space="DRAM") as drpool,
            tc.tile_pool(name="ps_big", bufs=6, space="PSUM") as ps_big,
            tc.tile_pool(name="ps_small", bufs=2, space="PSUM") as ps_small,
        ):
            # ---- constants ----
            ones_col = cpool.tile([128, 1], BF)
            nc.vector.memset(ones_col[:], 1.0)
            ones_row = cpool.tile([1, 128], BF)
            nc.vector.memset(ones_row[:], 1.0)
            one11 = cpool.tile([1, 1], F32)
            nc.vector.memset(one11[:], 1.0)

            qmi = cpool.tile([128, 4, 512], F32)
            ssuf = cpool.tile([1, 8 * D], BF)

            # persistent activations
            xT = actpool.tile([128, NDT, T], SDT, tag="xt")
            yT = actpool.tile([128, NDT, 1024], SDT, tag="yt")
            V = actpool.tile([128, NKT_ALL, D], BF, tag="v")
            pT = [
                actpool.tile([128, _wsc(g), 256], BF, tag=f"pt{g}", name=f"pt{g}")
                for g in range(4)
            ]

            # DRAM bounce buffers for the V collective (split into 2 AGs so
            # readback of early key tiles starts sooner)
            vbounce = [
                drpool.tile([128, 4 * D], BF, name=f"vbounce{h}") for h in range(2)
            ]
            vg = [drpool.tile([256, 4 * D], BF, name=f"vg{h}") for h in range(2)]



            # ---- phase A: V projection (own half), collective, yT ----
            with (
                tc.tile_pool(name="xpool", bufs=1) as xpool,
                tc.tile_pool(name="wpool", bufs=1) as wpool,
                tc.tile_pool(name="stpool", bufs=6) as stpool,
                tc.tile_pool(name="mpool", bufs=3) as mpool,
                tc.tile_pool(name="spool", bufs=2) as spool,
                tc.tile_pool(name="opool", bufs=3) as opool,
            ):
                wv_t = wpool.tile([128, NDT, D], BF, tag="wv")
                xh_t = xpool.tile([128, NDT, HALF], BF, tag="xh")
                # sync queue: wv ec0-half, xh hi-half, then xT q0/q1
                for dt in range(NDT):
                    nc.sync.dma_start(
                        wv_t[:, dt, 0:512],
                        wv_d.ap()[128 * dt : 128 * (dt + 1), 0:512],
                    )
                for dt in range(NDT):
                    nc.sync.dma_start(
                        xh_t[:, dt, 512:1024],
                        xTh[128 * dt : 128 * (dt + 1), 512:1024],
                    )
                for q4 in range(4):
                    for dt in range(NDT):
                        nc.sync.dma_start(
                            xT[:, dt, 512 * q4 : 512 * (q4 + 1)],
                            xT_ap[128 * dt : 128 * (dt + 1), 512 * q4 : 512 * (q4 + 1)],
                        )
                # gpsimd queue head is idle until the first bounce write:
                # land the small mask/suffix inputs + wv ec1-half there
                for g in range(4):
                    nc.gpsimd.dma_start(qmi[:, g, :], qmi_ap[g])
                nc.gpsimd.dma_start(ssuf[:], ssuf_d.ap())
                for dt in range(NDT):
                    nc.gpsimd.dma_start(
                        wv_t[:, dt, 512:1024],
                        wv_d.ap()[128 * dt : 128 * (dt + 1), 512:1024],
                    )
                # scalar queue: xh lo-half (first V chains), Mt + xq
                Mt = wpool.tile([128, NDT, D], BF, tag="m")
                xq_t = xpool.tile([128, NDT, 1024], BF, tag="xq")
                for dt in range(NDT):
                    nc.scalar.dma_start(
                        xh_t[:, dt, 0:512], xTh[128 * dt : 128 * (dt + 1), 0:512]
                    )
                for dt in range(NDT):
                    nc.scalar.dma_start(
                        Mt[:, dt, :], M_d.ap()[128 * dt : 128 * (dt + 1), :]
                    )
                    nc.scalar.dma_start(
                        xq_t[:, dt, 0:512], xTq[128 * dt : 128 * (dt + 1), 0:512]
                    )
                for dt in range(NDT):
                    nc.scalar.dma_start(
                        xq_t[:, dt, 512:1024],
                        xTq[128 * dt : 128 * (dt + 1), 512:1024],
                    )

                # V own half -> bounce (gpsimd queue, which is otherwise idle,
                # so AG#1 fires as soon as the ec0 chains complete)
                for ec in range(2):
                    for i in range(8):
                        ps = ps_big.tile([128, 512], F32, tag="big", name="ps")
                        for dt in range(NDT):
                            nc.tensor.matmul(
                                ps[:],
                                xh_t[:, dt, 128 * i : 128 * (i + 1)],
                                wv_t[:, dt, 512 * ec : 512 * (ec + 1)],
                                start=(dt == 0),
                                stop=(dt == NDT - 1),
                            )
                        st = stpool.tile([128, 512], BF, tag="st", name="st")
                        nc.vector.tensor_copy(st[:], ps[:])
                        nc.gpsimd.dma_start(
                            vbounce[ec][:, 512 * i : 512 * (i + 1)],
                            st[:],
                        )
                    nc.gpsimd.collective_compute(
                        "AllGather",
                        mybir.AluOpType.bypass,
                        replica_groups=GROUPS,
                        ins=[vbounce[ec].opt()],
                        outs=[vg[ec].opt()],
                    )
                # readback gathered V (ec0 on sync, ec1 on scalar; asc. kt)
                for ec in range(2):
                    q = nc.sync if ec == 0 else nc.scalar
                    for kt in range(NKT_ALL):
                        h, i = kt // 8, kt % 8
                        q.dma_start(
                            V[:, kt, 512 * ec : 512 * (ec + 1)],
                            vg[ec][
                                128 * h : 128 * (h + 1), 512 * i : 512 * (i + 1)
                            ],
                        )

                # yT = M^T x for own queries, c-major so the first score
                # groups (which read yT columns 0:512) unblock early
                def yt_half(c):
                    for et in range(NDT):
                        ps = ps_big.tile([128, 512], F32, tag="big", name="ps")
                        for dt in range(NDT):
                            nc.tensor.matmul(
                                ps[:],
                                Mt[:, dt, 128 * et : 128 * (et + 1)],
                                xq_t[:, dt, 512 * c : 512 * (c + 1)],
                                start=(dt == 0),
                                stop=(dt == NDT - 1),
                            )
                        nc.vector.tensor_copy(yT[:, et, 512 * c : 512 * (c + 1)], ps[:])

                # ---- phase B: scores (grouped), denominators, PV ----
                rcols = {}

                def scores_group(g):
                    for kt in range(_wsc(g)):
                        zpsA = ps_big.tile([128, 512], F32, tag="big", name="zps")
                        zps = zpsA[:, 0:256]
                        for d2 in range(NDT // 2):
                            nc.tensor.matmul(
                                zps,
                                xT[:, 2 * d2 : 2 * d2 + 2, 128 * kt : 128 * (kt + 1)],
                                yT[:, 2 * d2 : 2 * d2 + 2, 256 * g : 256 * (g + 1)],
                                start=(d2 == 0),
                                stop=(d2 == NDT // 2 - 1),
                                perf_mode=mybir.MatmulPerfMode.DoubleRow,
                            )
                        if kt >= 4 * g:
                            mt = mpool.tile([128, 256], F32, tag="mask", name="mt")
                            nc.vector.tensor_scalar(
                                mt[:],
                                qmi[:, g, 0:256],
                                float(128 * kt),
                                None,
                                op0=mybir.AluOpType.is_ge,
                            )
                            nc.vector.tensor_mul(zps, zps, mt[:])
                        nc.scalar.activation(
                            pT[g][:, kt, :],
                            zps,
                            Exp,
                            scale=1.0 / YSC,
                        )

                def den_rcol(o):
                    g, c = o // 2, o % 2
                    w = _w(o)
                    dps = ps_small.tile([1, 512], F32, tag="small", name="dps", bufs=1)
                    chunks = [(s, min(4, w - s)) for s in range(0, w, 4)]
                    for ci, (s, nk) in enumerate(chunks):
                        nc.tensor.matmul(
                            dps[0:1, 0 : 128 * nk],
                            ones_col[:],
                            pT[g][:, s : s + nk, 128 * c : 128 * (c + 1)],
                            start=(ci == 0),
                            stop=(ci == len(chunks) - 1),
                        )
                    nv = 128 * min(w, 4)
                    dsb = spool.tile([1, 512], F32, tag="dsb", name="dsb")
                    nc.vector.tensor_copy(dsb[0:1, 0:nv], dps[0:1, 0:nv])
                    t2 = spool.tile([1, 128], F32, tag="t2", name="t2")
                    if w == 2:
                        nc.vector.tensor_add(t2[:], dsb[0:1, 0:128], dsb[0:1, 128:256])
                    else:
                        t1 = spool.tile([1, 256], F32, tag="t1", name="t1")
                        nc.vector.tensor_add(t1[:], dsb[0:1, 0:256], dsb[0:1, 256:512])
                        nc.vector.tensor_add(t2[:], t1[0:1, 0:128], t1[0:1, 128:256])
                    drow = spool.tile([1, 128], F32, tag="drow", name="drow")
                    nc.vector.tensor_scalar_add(
                        drow[:], t2[:], float(T - 128 * w)
                    )
                    rrow = spool.tile([1, 128], F32, tag="rrow", name="rrow")
                    nc.vector.reciprocal(rrow[:], drow[:])
                    rps = ps_small.tile([128, 1], F32, tag="rden", name="rps", bufs=1)
                    nc.tensor.matmul(rps[:], rrow[0:1, :], one11[:])
                    rcol = spool.tile([128, 1], F32, tag="rcol", name="rcol", bufs=8)
                    nc.vector.tensor_copy(rcol[:], rps[:])
                    rcols[o] = rcol

                def pv_ec(o, ec):
                    g, c = o // 2, o % 2
                    w = _w(o)
                    nps = ps_big.tile([128, 512], F32, tag="big", name="nps")
                    for kt in range(w):
                        nc.tensor.matmul(
                            nps[:],
                            pT[g][:, kt, 128 * c : 128 * (c + 1)],
                            V[:, kt, 512 * ec : 512 * (ec + 1)],
                            start=(kt == 0),
                            stop=(kt == w - 1 and w == NKT_ALL),
                        )
                    if w < NKT_ALL:
                        nc.tensor.matmul(
                            nps[:],
                            ones_row[:],
                            ssuf[0:1, D * o + 512 * ec : D * o + 512 * (ec + 1)],
                            start=False,
                            stop=True,
                        )
                    ot = opool.tile([128, 512], BF, tag="out", name="ot")
                    nc.vector.tensor_scalar_mul(ot[:], nps[:], rcols[o][:])
                    nc.scalar.dma_start(
                        out_ap[
                            128 * o : 128 * (o + 1),
                            512 * ec : 512 * (ec + 1),
                        ],
                        ot[:],
                    )

                yt_half(0)
                scores_group(0)
                scores_group(1)
                yt_half(1)
                scores_group(2)
                scores_group(3)
                for o in range(8):
                    den_rcol(o)
                for o in range(8):
                    pv_ec(o, 0)
                for o in range(8):
                    pv_ec(o, 1)

    nc.compile()
    return nc


def get_nc():
    if "nc" not in _cache:
        _cache["nc"] = _build()
    return _cache["nc"]


def make_in_maps(x, Wq, Wk, Wv):
    x = np.asarray(x, np.float32)
    Wq32 = np.asarray(Wq, np.float32)
    Wk32 = np.asarray(Wk, np.float32)
    Wv32 = np.asarray(Wv, np.float32)

    msc = SCALE * YSC if FP8_SCORES else SCALE
    M_bf = ((Wq32.T @ Wk32) * np.float32(msc)).astype(bf16)
    wvT = np.ascontiguousarray(Wv32.T).astype(bf16)

    # qmi per parity: [4, 128, 512] f32; columns [0:256] hold q_glob - k,
    # columns [256:512] hold q_glob - k - 128 (the odd kt of the pair)
    qmis = []
    for p in range(2):
        q = np.empty((4, 128, 512), np.float32)
        for g in range(4):
            for half in range(2):
                sub = 4 * g + 2 * half + p
                q[g, :, 128 * half : 128 * (half + 1)] = (
                    128 * sub + np.arange(128, dtype=np.float32)
                )[None, :] - np.arange(128, dtype=np.float32)[:, None]
            q[g, :, 256:512] = q[g, :, 0:256] - 128.0
        qmis.append(q)

    # ssuf per batch: row o = colsum_{k >= 256(o+1)} V  (o=7 -> zeros)
    ssufs = []
    for b in range(B):
        rows = np.zeros((8, D), np.float32)
        for o in range(7):
            cs = x[b][256 * (o + 1) :, :].sum(axis=0, dtype=np.float32)
            rows[o] = cs @ Wv32.T
        ssufs.append(rows.reshape(1, 8 * D).astype(bf16))

    in_maps = []
    for core in range(N_CORES):
        b, p = core // 2, core % 2
        xt32 = np.ascontiguousarray(x[b].T)  # [D, T] f32
        xt = xt32.astype(f8e4 if FP8_SCORES else bf16)
        xtb = xt32.astype(bf16)
        xh = np.ascontiguousarray(xtb[:, HALF * p : HALF * (p + 1)])
        cols = [xtb[:, 128 * (2 * o + p) : 128 * (2 * o + p) + 128] for o in range(8)]
        xq = np.ascontiguousarray(np.concatenate(cols, axis=1))
        in_maps.append(
            {
                "xT": xt,
                "xTh": xh,
                "xTq": xq,
                "M": M_bf,
                "wv": wvT,
                "qmi": qmis[p],
                "ssuf": ssufs[b],
            }
        )
    return in_maps


def assemble(results):
    full = np.empty((B, T, D), np.float32)
    for core in range(N_CORES):
        b, p = core // 2, core % 2
        o_np = np.asarray(results[core]["out"], dtype=np.float32)
        for o in range(8):
            g = 2 * o + p
            full[b, 128 * g : 128 * (g + 1), :] = o_np[128 * o : 128 * (o + 1), :]
    return full


def kernel(x, Wq, Wk, Wv):
    global LAST_RESULT
    nc = get_nc()
    in_maps = make_in_maps(x, Wq, Wk, Wv)
    res = bass_utils.run_bass_kernel_spmd(nc, in_maps, core_ids=list(range(N_CORES)))
    LAST_RESULT = res
    return assemble(res.results)
